# revision 33
# baseline (speedup 1.0000x reference)
"""Trainium2 Bass kernel for ConcatBiInteraction — sin-factorized scores, v13.

Math: score s[n,l] = sum_k w2[k]*tanh(pa[k,l] + ab[k,n]) via a Q=3 sin-basis
fit of tanh (Gaussian-weighted, sigma=1, refit for the actual z-range):
  tanh(z) ~= sum_q A_q sin(w_q z);  the sin addition formula turns each term
into two rank-128 matmuls:  s ~= sum_q [fc1_q.T @ e1_q + fc2_q.T @ e2_q]
  e1 = sin(S2*t - 5pi/4), e2 = sin(S2*t - 3pi/4),
  t  = int32(pap * w_q/(2pi) * 2^18) & (2^18-1)    (exact angle mod 2pi)
with the ab-side phases folded into fc1/fc2 (same int trick + phase DQ_q).

Structure (one NeuronCore, 4 molecules, ~43us at full clock):
- tei ops read pa straight from a 4-bank PSUM tile (no SBUF copy), with the
  +C_PA*w fold in the tensor_scalar add slot; mol 2/3 half first (its protT
  half arrives on the scalar DMA ring first).
- three single-q groups: the per-q [128,2048] DVE->ACT->PE chains pipeline
  so the ACT engine runs its six [128,2048] Sin passes back to back.
- diagonal-block score matmuls: 32-column fc slabs at tile_position (0,32c)
  accumulate W[atom, l-of-own-molecule] into ONE PSUM bank; the prot-side
  segment max needs only 4 fp16 transposes + one grouped reduce.
- single ACT table (silu set: Sin+Tanh+Relu+Identity), preloaded explicitly;
  the two tiny exps use exp(y) = (1+tanh(y/2))/(1-tanh(y/2)) so no mid-kernel
  table swap exists.
- atom-side max runs on the raw PSUM scores (tanh is monotone) in parallel
  with the big tanh; [wc4 | expW] share one tile so sc+den is one matmul;
  1/norm broadcast via gpsimd partition_broadcast.
- fp16 everywhere on the PE (weights, e-fields, W, pools, MLP), fp32 PSUM.
- end-to-end rel err ~5.7e-3 vs the 2e-2 gate (CoreSim and hardware agree).

Sharding: 4 molecules per core x 8 cores, no cross-core communication.
"""

import numpy as np

import concourse.bass as bass
import concourse.tile as tile
from concourse import bacc, mybir
from concourse.bass_utils import run_bass_kernel_spmd

FP = mybir.dt.float32
F16 = mybir.dt.float16
I32 = mybir.dt.int32
AF = mybir.ActivationFunctionType
ALU = mybir.AluOpType

B, L, P, A = 32, 512, 128, 128
N = 1024
H1, H2 = 512, 256
NCORES = 8
MPC = B // NCORES          # molecules per core
PAD_NEG = -10.0

# sin-basis fit of tanh, Gaussian weight sigma=1 on [-6.5, 6.5] (Q=3 refit)
OMEGAS = [0.4166, 1.3697, 2.6755]
ACOEF = [1.21582, 0.25697, 0.04724]
QF = len(OMEGAS)
QGROUPS = [[0], [1], [2]]  # singles: tightest ACT/DVE pipelining
# ab-side int-trick range reduction (exact mod 2^18 via float->int + AND)
BITS = 18
PERIOD = 1 << BITS
MASK = PERIOD - 1
S2 = float(2 * np.pi / PERIOD)
TWO_PI = float(np.float32(2 * np.pi))
C_PA = 8.0
C_AB = 8.0
B5 = float(-5 * np.pi / 4)
B3 = float(-3 * np.pi / 4)
SCQ = [float(np.float32(om / (2 * np.pi) * PERIOD)) for om in OMEGAS]
DQ = [float(np.float32(np.mod(-om * (C_PA + C_AB) + np.pi / 2, 2 * np.pi)
                       / (2 * np.pi) * PERIOD)) for om in OMEGAS]

CL = MPC * L               # 2048 score columns (m, l)

# packed fp32 constant columns
C_B1 = 0        # att1 bias (1)
C_B2 = 1        # att2 bias broadcast (1)
C_FCS = 2       # A_q * w2 (QF)
C_SEG = C_FCS + QF          # slot->mol mask (MPC)
C_PAD = C_SEG + MPC         # 0 real / PAD_NEG pad (1)
C_D1B = C_PAD + 1           # d1 bias (4)
C_D2B = C_D1B + 4           # d2 bias (2)
C_BE5 = C_D2B + 2           # -5pi/4 bias column
C_BE3 = C_BE5 + 1           # -3pi/4 bias column
C_ZERO = C_BE3 + 1          # zero bias column
C_CPA = C_ZERO + 1          # +C_PA bias column
NC32 = C_CPA + 1
# packed fp16 columns
H_ONE = 0       # ones column (1)
H_ATOM = 1      # atom embeds per slot (A)
H_ID = H_ATOM + A           # identity (128)
NC16 = H_ID + 128

_cache: dict = {}


def _build(padded: bool):
    nc = bacc.Bacc("TRN2", target_bir_lowering=False, debug=False)

    def din(name, shape, dt=FP):
        return nc.dram_tensor(name, list(shape), dt, kind="ExternalInput").ap()

    wp_d = din("wp", [P, 128], F16)
    prot_T = din("prot_T", [MPC, P, L], F16)
    prot_N = din("prot_N", [MPC, L, P], F16)
    wa_d = din("wa", [A, 128])
    atom_T = din("atom_T", [A, 128])
    c32_d = din("c32", [128, NC32])
    c16_d = din("c16", [128, NC16], F16)
    row_d = din("rowp", [1, 130])
    dw_d = din("dw", [128, 2 * H1 + 4 * (H2 // 128) * 128 + 2], F16)
    out_d = nc.dram_tensor("out", [MPC, 1], FP, kind="ExternalOutput").ap()

    NJ1 = H1 // 128            # 4
    NJ2 = H2 // 128            # 2
    DW_D2 = 2 * H1             # d2 column base in dw pack
    DW_OW = DW_D2 + NJ1 * H2   # ow column base

    from contextlib import ExitStack
    with tile.TileContext(nc) as tc, ExitStack() as ctx:
        cpool = ctx.enter_context(tc.tile_pool(name="consts", bufs=1))
        tpool = ctx.enter_context(tc.tile_pool(name="tred", bufs=3))
        epool = ctx.enter_context(tc.tile_pool(name="etile", bufs=3))
        spool = ctx.enter_context(tc.tile_pool(name="smallsb", bufs=2))
        pp_s = ctx.enter_context(tc.tile_pool(name="sacc", bufs=1, space="PSUM"))
        pp_pa = ctx.enter_context(tc.tile_pool(name="pa", bufs=1, space="PSUM"))
        pp_mm = ctx.enter_context(tc.tile_pool(name="mm", bufs=2, space="PSUM"))
        pp_tp = ctx.enter_context(tc.tile_pool(name="tp", bufs=1, space="PSUM"))

        # preload the silu table (Sin+Tanh+Relu+Identity) once, up front,
        # so the auto-inserted per-function table swaps disappear
        nc.scalar.add_instruction(mybir.InstLoadActFuncSet(
            act_func_set_id=18, name=nc.get_next_instruction_name(),
            engine=mybir.EngineType.Activation))

        # ---- DMA: two hwdge queues, critical loads first.
        # wp is tiny: it rides the scalar ring ahead of protT half B while
        # protT half A leads the sync ring -> pa mm0 starts ~1.5us earlier.
        wp_sb = cpool.tile([128, 128], F16, tag="wp")
        nc.scalar.dma_start(wp_sb[:], wp_d[:])
        protT_sb = cpool.tile([128, CL], F16, tag="protT")
        nc.sync.dma_start(protT_sb[:, 0:CL // 2].rearrange("p (m l) -> p m l", m=2),
                          prot_T[0:2].rearrange("m p l -> p m l"))
        nc.scalar.dma_start(protT_sb[:, CL // 2:].rearrange("p (m l) -> p m l", m=2),
                          prot_T[2:4].rearrange("m p l -> p m l"))
        c32_sb = cpool.tile([128, NC32], FP, tag="c32")
        nc.scalar.dma_start(c32_sb[:], c32_d[:])
        wa_sb = cpool.tile([128, 128], FP, tag="wa")
        nc.sync.dma_start(wa_sb[:], wa_d[:])
        atomT_sb = cpool.tile([128, 128], FP, tag="atomT")
        nc.sync.dma_start(atomT_sb[:], atom_T[:])
        c16_sb = cpool.tile([128, NC16], F16, tag="c16")
        nc.scalar.dma_start(c16_sb[:], c16_d[:])
        dw_sb = cpool.tile([128, DW_OW + 2], F16, tag="dw")
        nc.scalar.dma_start(dw_sb[:], dw_d[:])
        row_sb = cpool.tile([1, 130], FP, tag="rowp")
        nc.scalar.dma_start(row_sb[:], row_d[:])
        protN_sb = cpool.tile([128, CL], F16, tag="protN")
        nc.sync.dma_start(protN_sb[:].rearrange("p (mc f) -> p mc f", mc=4 * MPC),
                            prot_N[:].rearrange("m (c p) f -> p (m c) f", c=4))

        # ---- pa matmuls; mol 0/1 first (sync protT half lands first) ----
        pa_ps = pp_pa.tile([128, CL], FP, tag="pa", name="pa_ps")
        for m in (0, 1, 2, 3):
            nc.tensor.matmul(pa_ps[:, m * L:(m + 1) * L], wp_sb[:],
                             protT_sb[:, m * L:(m + 1) * L],
                             skip_group_check=True)

        # ---- ab' = Wa.T @ atomT + b1 + C_AB  (fp32) ----
        ab_ps = pp_mm.tile([128, 128], FP, tag="mm")
        nc.tensor.matmul(ab_ps[:], wa_sb[:], atomT_sb[:])
        abp_sb = cpool.tile([128, 128], FP, tag="abp")

        # ---- main loop + F side, interleaved for engine overlap ----
        # tei reads pa straight from PSUM with the +C_PA*w fold in the add slot
        s2_ps = pp_s.tile([128, L], FP, tag="s2", name="s2_ps")
        fc1_sb = cpool.tile([128, QF * 128], F16, tag="fc1")
        fc2_sb = cpool.tile([128, QF * 128], F16, tag="fc2")
        f1_sb = cpool.tile([128, QF * 128], FP, tag="f1")
        f2_sb = cpool.tile([128, QF * 128], FP, tag="f2")
        abt_i = cpool.tile([128, QF * 128], I32, tag="abt")
        abt_r = cpool.tile([128, QF * 128], I32, tag="abtr")

        def emit_te(grp):
            ng = len(grp)
            te_i = tpool.tile([128, ng * CL], I32, tag="tei", name="te_i")
            for ql, q in enumerate(grp):
                for h in (0, 1):   # mol 0/1 half first — its pa lands first
                    nc.vector.tensor_scalar(
                        te_i[:, ql * CL + h * 1024:ql * CL + (h + 1) * 1024],
                        pa_ps[:, h * 1024:(h + 1) * 1024], SCQ[q],
                        C_PA * SCQ[q], ALU.mult, ALU.add)
            te_r = tpool.tile([128, ng * CL], I32, tag="ter", name="te_r")
            nc.vector.tensor_scalar(te_r[:], te_i[:], MASK, None,
                                    ALU.bitwise_and)
            return te_r

        def emit_sins(te_r, ng):
            e1p = epool.tile([128, ng * CL], F16, tag="e1", name="e1p")
            nc.scalar.activation(e1p[:], te_r[:], AF.Sin, scale=S2,
                                 bias=c32_sb[:, C_BE5:C_BE5 + 1])
            e2p = epool.tile([128, ng * CL], F16, tag="e2", name="e2p")
            nc.scalar.activation(e2p[:], te_r[:], AF.Sin, scale=S2,
                                 bias=c32_sb[:, C_BE3:C_BE3 + 1])
            return e1p, e2p

        # group 0 DVE + ACT first, then the F side, then groups 1/2
        ter0 = emit_te(QGROUPS[0])
        e1p0, e2p0 = emit_sins(ter0, len(QGROUPS[0]))

        # F side: DVE ops queued after group-0 te; ACT after group-0 sins;
        # the coefficient folds sit late on DVE (after ter2) off the chain
        nc.vector.tensor_scalar(abp_sb[:], ab_ps[:], c32_sb[:, C_B1:C_B1 + 1],
                                C_AB, ALU.add, ALU.add)
        for q in range(QF):
            nc.vector.tensor_scalar(abt_i[:, q * 128:(q + 1) * 128], abp_sb[:],
                                    SCQ[q], DQ[q], ALU.mult, ALU.add)
        nc.vector.tensor_scalar(abt_r[:], abt_i[:], MASK, None, ALU.bitwise_and)
        nc.scalar.activation(f1_sb[:], abt_r[:], AF.Sin, scale=S2,
                             bias=c32_sb[:, C_BE3:C_BE3 + 1])
        nc.scalar.activation(f2_sb[:], abt_r[:], AF.Sin, scale=S2,
                             bias=c32_sb[:, C_BE5:C_BE5 + 1])

        ter1 = emit_te(QGROUPS[1])
        e1p1, e2p1 = emit_sins(ter1, len(QGROUPS[1]))
        ter2 = emit_te(QGROUPS[2])

        for q in range(QF):
            nc.vector.tensor_scalar(fc1_sb[:, q * 128:(q + 1) * 128],
                                    f1_sb[:, q * 128:(q + 1) * 128],
                                    c32_sb[:, C_FCS + q:C_FCS + q + 1], None,
                                    ALU.mult)
            nc.vector.tensor_scalar(fc2_sb[:, q * 128:(q + 1) * 128],
                                    f2_sb[:, q * 128:(q + 1) * 128],
                                    c32_sb[:, C_FCS + q:C_FCS + q + 1], None,
                                    ALU.mult)

        e1p2, e2p2 = emit_sins(ter2, len(QGROUPS[2]))
        e_tiles = {0: (e1p0, e2p0), 1: (e1p1, e2p1), 2: (e1p2, e2p2)}
        for gi, grp in enumerate(QGROUPS):
            ng = len(grp)
            e1p, e2p = e_tiles[gi]
            for ql, q in enumerate(grp):
                for c in range(MPC):
                    nc.tensor.matmul(
                        s2_ps[32 * c:32 * (c + 1), :],
                        fc1_sb[:, q * 128 + 32 * c:q * 128 + 32 * (c + 1)],
                        e1p[:, ql * CL + c * L:ql * CL + (c + 1) * L],
                        start=(gi == 0 and ql == 0), stop=False,
                        tile_position=(0, 32 * c), skip_group_check=True)
            for ql, q in enumerate(grp):
                for c in range(MPC):
                    nc.tensor.matmul(
                        s2_ps[32 * c:32 * (c + 1), :],
                        fc2_sb[:, q * 128 + 32 * c:q * 128 + 32 * (c + 1)],
                        e2p[:, ql * CL + c * L:ql * CL + (c + 1) * L],
                        start=False,
                        stop=(gi == len(QGROUPS) - 1 and ql == ng - 1),
                        tile_position=(0, 32 * c), skip_group_check=True)

        # ---- atom-side max straight off the PSUM scores (tanh monotone),
        #      runs on DVE in parallel with the big tanh on ACT ----
        wcps_sb = spool.tile([128, 1], FP, tag="wcps")
        nc.vector.reduce_max(wcps_sb[:], s2_ps[:], axis=mybir.AxisListType.X)

        # ---- W = tanh(s + b2): [128 atom slots, 512 l-of-own-molecule] ----
        W2_sb = cpool.tile([128, L], F16, tag="W2")
        nc.scalar.activation(W2_sb[:], s2_ps[:], AF.Tanh,
                             bias=c32_sb[:, C_B2:C_B2 + 1])
        if padded:
            nc.vector.tensor_scalar_add(W2_sb[:], W2_sb[:],
                                        c32_sb[:, C_PAD:C_PAD + 1])

        def exp5(dst, src, n, tag):
            """dst = exp(5*src) via tanh identity (keeps the Sin ACT table)."""
            t = spool.tile([128, n], FP, tag=tag + "_t", name="exp_t")
            nc.scalar.activation(t[:], src, AF.Tanh, scale=2.5,
                                 bias=c32_sb[:, C_ZERO:C_ZERO + 1])
            u = spool.tile([128, n], FP, tag=tag + "_u", name="exp_u")
            nc.vector.tensor_scalar(u[:], t[:], -1.0, 1.0, ALU.mult, ALU.add)
            r = spool.tile([128, n], FP, tag=tag + "_r", name="exp_r")
            nc.vector.reciprocal(r[:], u[:])
            v = spool.tile([128, n], FP, tag=tag + "_v", name="exp_v")
            nc.vector.tensor_scalar(v[:], t[:], 1.0, None, ALU.add)
            nc.vector.tensor_tensor(dst, v[:], r[:], ALU.mult)

        # ---- atom side: tanh then exp via tanh identity (tiny) ----
        wct_sb = spool.tile([128, 1], FP, tag="wct")
        nc.scalar.activation(wct_sb[:], wcps_sb[:], AF.Tanh,
                             bias=c32_sb[:, C_B2:C_B2 + 1])
        if padded:
            nc.vector.tensor_scalar_add(wct_sb[:], wct_sb[:],
                                        c32_sb[:, C_PAD:C_PAD + 1])
        wc_sb = spool.tile([128, 1], F16, tag="wc")
        exp5(wc_sb[:], wct_sb[:], 1, "wce")
        tp_ps = pp_tp.tile([128, 4 * 128], F16, tag="tp", name="tp_ps")
        for cc in range(4):
            nc.tensor.transpose(tp_ps[:, cc * 128:(cc + 1) * 128],
                                W2_sb[:, cc * 128:(cc + 1) * 128],
                                c16_sb[:, H_ID:H_ID + 128])
        WpT_sb = spool.tile([128, 4 * MPC], FP, tag="WpT")
        nc.vector.reduce_max(WpT_sb[:],
                             tp_ps[:].rearrange("p (g j) -> p g j", j=32),
                             axis=mybir.AxisListType.X)
        wex_sb = spool.tile([128, 5 * MPC], F16, tag="wex")  # [wc4 | expW]
        nc.vector.tensor_mul(wex_sb[:, 0:MPC], c32_sb[:, C_SEG:C_SEG + MPC],
                             wc_sb[:, 0:1].to_broadcast([128, MPC]))
        exp5(wex_sb[:, MPC:5 * MPC], WpT_sb[:], 4 * MPC, "wpe")
        expW_sb = wex_sb[:, MPC:5 * MPC]
        wc4_sb = wex_sb[:, 0:MPC]

        ap_ps = pp_mm.tile([128, MPC], FP, tag="mm")
        nc.tensor.matmul(ap_ps[:], c16_sb[:, H_ATOM:H_ATOM + A], wc4_sb)
        den_ps = pp_mm.tile([1, 5 * MPC], FP, tag="mm")
        nc.tensor.matmul(den_ps[:], c16_sb[:, H_ONE:H_ONE + 1], wex_sb[:])
        nrm = spool.tile([1, 2 * MPC], FP, tag="nrm")
        nc.vector.tensor_copy(nrm[:, 0:MPC], den_ps[:, 0:MPC])
        nc.vector.reduce_sum(nrm[:, MPC:2 * MPC],
                             den_ps[:, MPC:5 * MPC].rearrange(
                                 "p (c m) -> p m c", m=MPC),
                             axis=mybir.AxisListType.X)
        rnrm = spool.tile([1, 2 * MPC], FP, tag="rnrm")
        nc.vector.reciprocal(rnrm[:], nrm[:])
        rb_sb = spool.tile([128, 2 * MPC], FP, tag="rb")
        nc.gpsimd.partition_broadcast(rb_sb[:], rnrm[:])

        pp_ps = pp_mm.tile([128, MPC], FP, tag="mm")
        for m in range(MPC):
            for cc in range(4):
                nc.tensor.matmul(pp_ps[:, m:m + 1],
                                 protN_sb[:, (m * 4 + cc) * 128:(m * 4 + cc + 1) * 128],
                                 expW_sb[:, cc * MPC + m:cc * MPC + m + 1],
                                 start=(cc == 0), stop=(cc == 3))

        apT_sb = spool.tile([128, MPC], F16, tag="apT")
        nc.vector.tensor_mul(apT_sb[:], ap_ps[:], rb_sb[:, 0:MPC])
        ppT_sb = spool.tile([128, MPC], F16, tag="ppT")
        nc.vector.tensor_mul(ppT_sb[:], pp_ps[:], rb_sb[:, MPC:2 * MPC])

        # ---- output MLP (fp16, molecules on the free axis) ----
        h1_sb = spool.tile([128, NJ1 * MPC], F16, tag="h1")
        for j in range(NJ1):
            h1_ps = pp_mm.tile([128, MPC], FP, tag="mm")
            nc.tensor.matmul(h1_ps[:], dw_sb[:, j * 128:(j + 1) * 128],
                             apT_sb[:], start=True, stop=False)
            nc.tensor.matmul(h1_ps[:], dw_sb[:, H1 + j * 128:H1 + (j + 1) * 128],
                             ppT_sb[:], start=False, stop=True)
            nc.scalar.activation(h1_sb[:, j * MPC:(j + 1) * MPC], h1_ps[:],
                                 AF.Relu, bias=c32_sb[:, C_D1B + j:C_D1B + j + 1])
        h2_sb = spool.tile([128, NJ2 * MPC], F16, tag="h2")
        for i in range(NJ2):
            h2_ps = pp_mm.tile([128, MPC], FP, tag="mm")
            for j in range(NJ1):
                nc.tensor.matmul(h2_ps[:],
                                 dw_sb[:, DW_D2 + j * H2 + i * 128:DW_D2 + j * H2 + (i + 1) * 128],
                                 h1_sb[:, j * MPC:(j + 1) * MPC],
                                 start=(j == 0), stop=(j == NJ1 - 1))
            nc.scalar.activation(h2_sb[:, i * MPC:(i + 1) * MPC], h2_ps[:],
                                 AF.Relu, bias=c32_sb[:, C_D2B + i:C_D2B + i + 1])
        o_ps = pp_mm.tile([1, MPC], FP, tag="mm")
        for i in range(NJ2):
            nc.tensor.matmul(o_ps[:], dw_sb[:, DW_OW + i:DW_OW + i + 1],
                             h2_sb[:, i * MPC:(i + 1) * MPC],
                             start=(i == 0), stop=(i == NJ2 - 1))
        o_sb = spool.tile([1, MPC], FP, tag="o")
        nc.scalar.activation(o_sb[:], o_ps[:], AF.Identity,
                             bias=row_sb[0:1, 128:129])
        nc.sync.dma_start(out_d[:], o_sb[0:1, :])

    nc.compile()
    return nc


def prepare(atom_embed, protSeq_embed, atom_splits,
            att1_W, att1_b, att2_W, att2_b,
            d1_W, d1_b, d2_W, d2_b, out_W, out_b):
    atom_embed = np.ascontiguousarray(atom_embed, dtype=np.float32)
    protSeq_embed = np.ascontiguousarray(protSeq_embed, dtype=np.float32)
    splits = np.asarray(atom_splits).astype(np.int64)
    assert atom_embed.shape == (N, A) and protSeq_embed.shape == (B, L, P)

    counts = np.bincount(splits, minlength=B)
    starts = np.concatenate([[0], np.cumsum(counts)])[:B]
    assert counts.max() <= 32, "fast path requires <=32 atoms per molecule"
    cap = 32
    padded = bool((counts != cap).any())

    key = ("sin4", padded)
    if key not in _cache:
        _cache[key] = _build(padded)
    nc = _cache[key]

    f32 = np.float32
    f16 = np.float16
    w2 = np.asarray(att2_W, f32)[:, 0]

    c32 = np.zeros((128, NC32), f32)
    c32[:, C_B1] = np.asarray(att1_b, f32)
    c32[:, C_B2] = np.asarray(att2_b, f32)[0]
    for q in range(QF):
        c32[:, C_FCS + q] = ACOEF[q] * w2
    c32[:, C_D1B:C_D1B + 4] = np.asarray(d1_b, f32).reshape(4, 128).T
    c32[:, C_D2B:C_D2B + 2] = np.asarray(d2_b, f32).reshape(2, 128).T
    c32[:, C_BE5] = B5
    c32[:, C_BE3] = B3
    c32[:, C_ZERO] = 0.0
    c32[:, C_CPA] = C_PA

    # dw pack: d1 (2*512), d2 (4*256), ow (2)
    d1r = np.asarray(d1_W, f32).reshape(2, 128, H1)
    d1cols = np.concatenate([d1r[0], d1r[1]], axis=1)            # [128, 1024]
    d2r = np.asarray(d2_W, f32).reshape(4, 128, H2)
    d2cols = np.concatenate(list(d2r), axis=1)                   # [128, 1024]
    owr = np.asarray(out_W, f32).reshape(2, 128).T               # [128, 2]
    dw = np.concatenate([d1cols, d2cols, owr], axis=1).astype(f16)

    rowp = np.zeros((1, 130), f32)
    rowp[0, 0:128] = 1.0
    rowp[0, 128] = np.asarray(out_b, f32)[0]

    shared = {
        "wp": np.ascontiguousarray(att1_W[:P], f32).astype(f16),
        "wa": np.ascontiguousarray(att1_W[P:], f32),
        "dw": dw,
        "rowp": rowp,
    }

    in_maps = []
    for cidx in range(NCORES):
        gm = range(MPC * cidx, MPC * (cidx + 1))
        aN = np.zeros((128, A), f32)
        c32c = c32.copy()
        c32c[:, C_PAD] = PAD_NEG
        for lm, g in enumerate(gm):
            cnt = int(counts[g])
            s0 = lm * cap
            aN[s0:s0 + cnt] = atom_embed[starts[g]:starts[g] + cnt]
            c32c[s0:s0 + cnt, C_SEG + lm] = 1.0
            c32c[s0:s0 + cnt, C_PAD] = 0.0
        c16 = np.zeros((128, NC16), f16)
        c16[:, H_ONE] = 1.0
        c16[:, H_ATOM:H_ATOM + A] = aN.astype(f16)
        c16[:, H_ID:H_ID + 128] = np.eye(128, dtype=f16)
        pmc = protSeq_embed[MPC * cidx:MPC * (cidx + 1)]
        in_maps.append({
            **shared,
            "c32": c32c,
            "c16": c16,
            "prot_T": np.ascontiguousarray(pmc.transpose(0, 2, 1)).astype(f16),
            "prot_N": np.ascontiguousarray(pmc).astype(f16),
            "atom_T": np.ascontiguousarray(aN.T),
        })

    return nc, in_maps


def kernel(**inputs):
    nc, in_maps = prepare(**inputs)
    res = run_bass_kernel_spmd(nc, in_maps, list(range(NCORES)))
    return np.concatenate([res.results[c]["out"] for c in range(NCORES)], axis=0)

